# revision 1
# baseline (speedup 1.0000x reference)
"""DMI-CE loss kernel for Trainium2 (8 NeuronCores, data-parallel over batch).

Problem: pred [256, 4, 16384] f32 logits, labels [256, 16384] i32 in {0,1,2,3}
(3 = pad/ignore).  Loss = 0.1 * mean_b(dmi_b) + CE where
  CE    = -(sum_valid logsoftmax(pred)[y]) / n_valid
  dmi_b = -sign(det(mat_b)) * log(|det(mat_b)| + 1e-3)
  mat_b = onehot(y)^T @ softmax(pred[:, :3]) / j_b   (over the valid prefix)

Sharding: pure data parallel, 32 samples per core.  Each core streams its
10 MiB shard once and reduces everything to a [128, 64] f32 accumulator
(per-(sample,quarter) partial dot products).  Host combines the 8 tiny
accumulators: per-sample 3x3 dets in f64, CE ratio, final scalar.

Layout on core: partition p = b_local*4 + hi  (hi = which quarter of the
token axis), free dim = (class, token-in-chunk).  All class arithmetic is
free-dim strided.  Masked token reductions use the exp-mask trick
(sum eq_c*q_d = sum exp((P_d - ln s3) + M_c), M_c = -1e30 off-class) so
every reduction rides the scalar engine's fused accum_out; counts come
from sum exp(M_c), and mat[:, 2] is reconstructed from row counts.
"""

import numpy as np

import concourse.bass as bass
import concourse.bacc as bacc
import concourse.tile as tile
from concourse import mybir
from concourse.bass_utils import run_bass_kernel_spmd

N_CORES = 8
B, C, L = 256, 4, 16384
B_LOC = B // N_CORES  # 32 samples per core
HI = 4                # token-axis quarters per sample; partition p = b*HI + hi
M = L // HI           # 4096 tokens per partition row
FC = 1024             # tokens per chunk
NCHUNK = M // FC      # 4
NQ = 16               # accumulator columns per chunk (14 used)
ACC_W = NCHUNK * NQ   # 64

IGNORE = 3
DMICE_P = 0.1

# test.py toggles TRACE to get exec_time_ns out of the NTFF profile.
TRACE = False
LAST_EXEC_NS = None
LAST_TRACE_PATH = None

_CACHE = {}


NEG = -1e30  # mask value; exp(NEG + anything finite) underflows to exactly 0

ACT_SET = "natural_log_exp_and_others"  # holds Exp, Ln, Copy, Identity


class _Bacc(bacc.Bacc):
    """Bacc whose act-table pass sees only one (correctly-indexed) set.

    The stock pass resolves each activation to the first table set
    containing its function, which ping-pongs Exp<->Ln between different
    sets and inserts a ~2.7us ACT_TABLE_LOAD per transition (17 loads =
    ~46us here).  All functions this kernel uses live together in
    natural_log_exp_and_others, so present every other set as empty; ids
    stay positional, so the emitted act_func_set_id is unchanged.
    """

    def insert_act_table_loads(self):
        from concourse import mybir as _mb
        from concourse.hw_specs import get_activation_tables
        import bass_rust as _bass_rust
        has_activation = any(
            isinstance(i, _mb.InstActivation)
            for b in self.main_func.blocks
            for i in b.instructions
        )
        if not has_activation:
            return
        tables = [
            (name, funcs if name == ACT_SET else set())
            for name, funcs in get_activation_tables(self.m.arch).items()
        ]
        _bass_rust.insert_act_table_loads(self, tables)


def _build():
    f32 = mybir.dt.float32
    i32 = mybir.dt.int32
    Alu = mybir.AluOpType
    Act = mybir.ActivationFunctionType

    nc = _Bacc("TRN2", debug=False, target_bir_lowering=False,
               num_devices=N_CORES)
    pred_d = nc.dram_tensor("pred", [B_LOC, C, L], f32, kind="ExternalInput")
    lab_d = nc.dram_tensor("labels", [B_LOC, L], i32, kind="ExternalInput")
    acc_d = nc.dram_tensor("acc", [128, ACC_W], f32, kind="ExternalOutput")

    # 4-D DRAM APs iterated b -> hi -> c -> m; SBUF side is [128=(b,hi), ...]
    # in the same order, so a plain dma_start matches element-for-element.
    pred_v = pred_d.ap().rearrange("b c (h m) -> b h c m", h=HI)
    lab_v = lab_d.ap().rearrange("b (h m) -> b h m", h=HI)

    # Accumulator column map (per chunk block of NQ):
    #   0..5:  m[c][d] = sum_l eq_c * q_d   for c in 0..2, d in 0..1 (c*2+d)
    #   6..8:  n_c     = sum_l eq_c         (counts; m[c][2] = n_c - m[c][0]
    #                                        - m[c][1], j = n0+n1+n2)
    #   9..11: pk_c    = sum_l eq_c * pred_c
    #   12:    vl      = sum_l valid * log(s4)
    with tile.TileContext(nc) as tc:
        with (
            tc.tile_pool(name="io", bufs=2) as io_pool,
            tc.tile_pool(name="ec", bufs=3) as ec_pool,
            tc.tile_pool(name="work", bufs=2) as work_pool,
            tc.tile_pool(name="roll", bufs=3) as roll_pool,
            tc.tile_pool(name="ds", bufs=6) as ds_pool,
            tc.tile_pool(name="scrp", bufs=1) as scr_pool,
            tc.tile_pool(name="accp", bufs=1) as acc_pool,
        ):
            acc = acc_pool.tile([128, ACC_W], f32)
            nc.vector.memzero(acc[:])

            for k in range(NCHUNK):
                base = k * NQ
                pt = io_pool.tile([128, C * FC], f32, tag="pt")
                yt = io_pool.tile([128, FC], i32, tag="yt")
                for c in range(C):
                    nc.sync.dma_start(
                        out=pt[:, c * FC:(c + 1) * FC],
                        in_=pred_v[:, :, c, k * FC:(k + 1) * FC])
                nc.sync.dma_start(out=yt[:],
                                  in_=lab_v[:, :, k * FC:(k + 1) * FC])

                def col(t, c):
                    return t[:, c * FC:(c + 1) * FC]

                # exp per class into small rotating tiles; only the running
                # class-sum survives (s3 = e0+e1+e2, s4 = s3+e3)
                ecs = []
                for c in range(C):
                    ec = ec_pool.tile([128, FC], f32, tag="ec")
                    nc.scalar.activation(ec[:], col(pt, c), Act.Exp)
                    ecs.append(ec)
                s01 = work_pool.tile([128, FC], f32, tag="s01")
                s3 = work_pool.tile([128, FC], f32, tag="s3")
                s4 = work_pool.tile([128, FC], f32, tag="s4")
                nc.vector.tensor_add(s01[:], ecs[0][:], ecs[1][:])
                nc.vector.tensor_add(s3[:], s01[:], ecs[2][:])
                nc.vector.tensor_add(s4[:], s3[:], ecs[3][:])

                ln3 = work_pool.tile([128, FC], f32, tag="ln3")
                nc.scalar.activation(ln3[:], s3[:], Act.Ln)

                # log softmax3 numerators: ln q_d = pred_d - ln(s3)
                lnq = []
                for d in range(2):
                    lq = work_pool.tile([128, FC], f32, tag=f"lnq{d}")
                    nc.vector.tensor_sub(lq[:], col(pt, d), ln3[:])
                    lnq.append(lq)

                # CE numerator: vl = sum valid*ln(s4) = sum ln((s4-1)*valid+1)
                vld = work_pool.tile([128, FC], f32, tag="vld")
                nc.vector.tensor_scalar(vld[:], yt[:], float(IGNORE), None,
                                        Alu.is_lt)
                s4b = work_pool.tile([128, FC], f32, tag="s4b")
                nc.vector.tensor_scalar(s4b[:], s4[:], 1.0, None, Alu.subtract)
                nc.vector.tensor_mul(s4b[:], s4b[:], vld[:])
                nc.scalar.activation(s4b[:], s4b[:], Act.Ln, bias=1.0,
                                     accum_out=acc[:, base + 12:base + 13])

                # scratch for ACT instructions whose elementwise output is
                # unused (only accum_out matters); ACT is serial so WAW is free
                scr = scr_pool.tile([128, FC], f32, tag="scr")
                for c in range(3):
                    # M_c = (y != c) * NEG   (0 on tokens of class c, else -inf)
                    mc = roll_pool.tile([128, FC], f32, tag="mc")
                    nc.vector.tensor_scalar(mc[:], yt[:], float(c), NEG,
                                            Alu.not_equal, Alu.mult)
                    # eq_c = exp(M_c) in {0,1}; accum gives n_c for free
                    eqc = roll_pool.tile([128, FC], f32, tag="eqc")
                    nc.scalar.activation(
                        eqc[:], mc[:], Act.Exp,
                        accum_out=acc[:, base + 6 + c:base + 7 + c])
                    # DMI entries: sum exp(lnq_d + M_c) = sum eq_c * q_d
                    for d in range(2):
                        ds = ds_pool.tile([128, FC], f32, tag="ds")
                        nc.vector.tensor_add(ds[:], lnq[d][:], mc[:])
                        nc.scalar.activation(
                            scr[:], ds[:], Act.Exp,
                            accum_out=acc[:, base + 2 * c + d:
                                          base + 2 * c + d + 1])
                    # CE picked logits: pk_c = sum eq_c * pred_c
                    tk = roll_pool.tile([128, FC], f32, tag="tk")
                    nc.vector.tensor_mul(tk[:], eqc[:], col(pt, c))
                    nc.scalar.activation(
                        scr[:], tk[:], Act.Copy,
                        accum_out=acc[:, base + 9 + c:base + 10 + c])

            nc.sync.dma_start(out=acc_d.ap(), in_=acc[:])
    nc.compile()
    return nc


def _get_nc():
    if "nc" not in _CACHE:
        _CACHE["nc"] = _build()
    return _CACHE["nc"]


def _finalize(acc_list):
    """acc_list: per-core [128, ACC_W] f32 -> scalar loss (f64 host math)."""
    per_sample = []
    for a in acc_list:
        a = a.astype(np.float64).reshape(128, NCHUNK, NQ).sum(axis=1)
        a = a.reshape(B_LOC, HI, NQ).sum(axis=1)  # [32, NQ]
        per_sample.append(a)
    a = np.concatenate(per_sample, axis=0)  # [256, NQ]
    m01 = a[:, 0:6].reshape(B, 3, 2)        # mat[:, c, 0:2] unnormalized
    n_c = a[:, 6:9]                         # per-class valid-token counts
    mat_u = np.concatenate(
        [m01, (n_c - m01.sum(axis=2))[:, :, None]], axis=2)  # [B, 3, 3]
    pk_total = a[:, 9:12].sum()
    vl_total = a[:, 12].sum()
    j = n_c.sum(axis=1)
    mat = mat_u / j[:, None, None]
    det = np.linalg.det(mat)
    dmi = np.where(det < 0, np.log(np.abs(det) + 1e-3),
                   -np.log(np.abs(det) + 1e-3))
    ce = (vl_total - pk_total) / j.sum()
    loss = DMICE_P * (dmi.sum() / B) + ce
    return np.asarray(loss, dtype=np.float32)


def kernel(pred, labels):
    global LAST_EXEC_NS, LAST_TRACE_PATH
    pred = np.asarray(pred, dtype=np.float32)
    labels = np.asarray(labels, dtype=np.int32)
    assert pred.shape == (B, C, L) and labels.shape == (B, L)
    nc = _get_nc()
    in_maps = [
        {
            "pred": np.ascontiguousarray(pred[i * B_LOC:(i + 1) * B_LOC]),
            "labels": np.ascontiguousarray(labels[i * B_LOC:(i + 1) * B_LOC]),
        }
        for i in range(N_CORES)
    ]
    res = run_bass_kernel_spmd(nc, in_maps, core_ids=list(range(N_CORES)),
                               trace=TRACE)
    LAST_EXEC_NS = res.exec_time_ns
    if res.instructions_and_trace is not None:
        LAST_TRACE_PATH = res.instructions_and_trace[1]
    return _finalize([r["acc"] for r in res.results])


if __name__ == "__main__":
    nc = _build()
    print("build ok")



# revision 2
# speedup vs baseline: 1.0774x; 1.0774x over previous
"""DMI-CE loss kernel for Trainium2 (8 NeuronCores, data-parallel over batch).

Problem: pred [256, 4, 16384] f32 logits, labels [256, 16384] i32 in {0,1,2,3}
(3 = pad/ignore).  Loss = 0.1 * mean_b(dmi_b) + CE where
  CE    = -(sum_valid logsoftmax(pred)[y]) / n_valid
  dmi_b = -sign(det(mat_b)) * log(|det(mat_b)| + 1e-3)
  mat_b = onehot(y)^T @ softmax(pred[:, :3]) / j_b   (over the valid prefix)

Layout: dense (b, hi)-partitions (32 samples x 4 token-quarters = 128), free
dim = (class, token-in-chunk).  Engine split per chunk:
  ACT : exp(P)->bf16 (one 4-plane instr), Ln(s3'), r3=exp(-Ln s3'), Ln(s4)
        with accum (sum of ln s4 over ALL tokens).
  PE  : s3' = e0+e1+e2 + BIG*eq3 and s4 = e0+..+e3 via identity-stationary
        accumulating matmuls into PSUM (bf16 rhs, 1 cyc/row).
  DVE : label cast, eq masks (tensor_scalar 4x + free accum -> counts),
        q_d = e_d*r3 (bf16 2x), masked products eq_c*q_d (2x), and all
        reductions via tensor_scalar accum (4x).
  Pool: picked-logit products eq_c*pred_c and the pad-side ce product
        eq3*ln(s4) (otherwise idle engine).
Only mat[c,d] for c,d in {0,1} is measured directly; row/col sums (counts
n_c and column totals t_d = sum_valid q_d) reconstruct the rest:
  mat[c,2] = n_c - mat[c,0] - mat[c,1],  mat[2,d] = t_d - mat[0,d] - mat[1,d].
CE numerator = (sum_all ln s4 - sum_pad ln s4) - sum_valid p_y.
Host combines the per-core [128, 48] f32 accumulators in f64.
"""

import numpy as np

import concourse.bass as bass
import concourse.bacc as bacc
import concourse.tile as tile
from concourse import mybir
from concourse import masks as _masks
from concourse.bass_utils import run_bass_kernel_spmd

N_CORES = 8
B, C, L = 256, 4, 16384
B_LOC = B // N_CORES   # 32 samples per core
HI = 4                 # token-axis quarters; partition p = b*HI + hi
M = L // HI            # 4096 tokens per partition row
FC = 1024              # tokens per chunk (per partition row)
NCHUNK = M // FC       # 4
NQ = 12                # accumulator columns per chunk
ACC_W = NCHUNK * NQ    # 48
MMW = 512              # PSUM-bank-sized matmul output slice

IGNORE = 3
DMICE_P = 0.1
BIG = float(2 ** 27)   # pad-token additive mask; exact in bf16

# test.py toggles TRACE to get exec_time_ns out of the NTFF profile.
TRACE = False
LAST_EXEC_NS = None
LAST_TRACE_PATH = None

_CACHE = {}

ACT_SET = "natural_log_exp_and_others"  # holds Exp, Ln, Copy, Identity


class _Bacc(bacc.Bacc):
    """Bacc whose act-table pass sees only one (correctly-indexed) set.

    The stock pass resolves each activation to the first table set
    containing its function, which can ping-pong between sets and insert
    ~1.3us ACT_TABLE_LOADs.  All functions this kernel uses live together
    in natural_log_exp_and_others, so present every other set as empty;
    ids stay positional, so the emitted act_func_set_id is unchanged.
    """

    def insert_act_table_loads(self):
        from concourse import mybir as _mb
        from concourse.hw_specs import get_activation_tables
        import bass_rust as _bass_rust
        has_activation = any(
            isinstance(i, _mb.InstActivation)
            for b in self.main_func.blocks
            for i in b.instructions
        )
        if not has_activation:
            return
        tables = [
            (name, funcs if name == ACT_SET else set())
            for name, funcs in get_activation_tables(self.m.arch).items()
        ]
        _bass_rust.insert_act_table_loads(self, tables)


def _build():
    f32 = mybir.dt.float32
    bf16 = mybir.dt.bfloat16
    i32 = mybir.dt.int32
    Alu = mybir.AluOpType
    Act = mybir.ActivationFunctionType

    nc = _Bacc("TRN2", debug=False, target_bir_lowering=False,
               num_devices=N_CORES)
    pred_d = nc.dram_tensor("pred", [B_LOC, C, L], f32, kind="ExternalInput")
    lab_d = nc.dram_tensor("labels", [B_LOC, L], i32, kind="ExternalInput")
    acc_d = nc.dram_tensor("acc", [128, ACC_W], f32, kind="ExternalOutput")

    # DRAM views matching the (b, hi) partition layout.
    pred_v = pred_d.ap().rearrange("b c (h m) -> b h c m", h=HI)
    lab_v = lab_d.ap().rearrange("b (h m) -> b h m", h=HI)

    # Accumulator column map (per chunk block of NQ):
    #   0..2 : n_c   = sum eq_c            (c = 0,1,2)
    #   3..4 : t_d   = sum q_d             (valid only; r3 masks pads)
    #   5..8 : m_cd  = sum eq_c * q_d      ((c,d) in {0,1}^2: 00,01,10,11)
    #   9    : pk    = sum_c sum eq_c * p_c
    #   10   : ceS   = sum_all ln(s4 + BIG*eq3)
    #                  (= sum_valid ln s4 + npad*ln(BIG) up to ~1e-6 rel)
    with tile.TileContext(nc) as tc:
        with (
            tc.tile_pool(name="io", bufs=4) as io_pool,
            tc.tile_pool(name="msk", bufs=3) as msk_pool,
            tc.tile_pool(name="qp", bufs=3) as q_pool,
            tc.tile_pool(name="scr", bufs=4) as scr_pool,
            tc.tile_pool(name="ps", bufs=4, space="PSUM") as ps_pool,
            tc.tile_pool(name="st", bufs=1) as st_pool,
        ):
            acc = st_pool.tile([128, ACC_W], f32)
            nc.vector.memzero(acc[:])

            ident = st_pool.tile([128, 128], bf16)
            _masks.make_identity(nc, ident[:])
            identB = st_pool.tile([128, 128], bf16)
            nc.vector.tensor_scalar(identB[:], ident[:], BIG, None, Alu.mult)

            def col(k, i):
                return acc[:, k * NQ + i:k * NQ + i + 1]

            stash = {}

            def stage_a(k):
                # DMA, masks, exp, PE class sums (into PSUM)
                ks = slice(k * FC, (k + 1) * FC)
                yt = io_pool.tile([128, FC], i32, tag="yt")
                nc.sync.dma_start(out=yt[:], in_=lab_v[:, :, ks])
                pt = io_pool.tile([128, C, FC], f32, tag="pt")
                for c in range(C):
                    nc.sync.dma_start(out=pt[:, c, :],
                                      in_=pred_v[:, :, c, ks])

                yf = msk_pool.tile([128, FC], bf16, tag="yf")
                nc.vector.tensor_scalar(yf[:], yt[:], 1.0, None, Alu.mult)
                eqs = []
                for c in range(3):
                    eq = msk_pool.tile([128, FC], bf16, tag=f"eq{c}")
                    nc.vector.tensor_scalar(eq[:], yf[:], float(c), 0.0,
                                            Alu.is_equal, Alu.add,
                                            accum_out=col(k, c))
                    eqs.append(eq)
                eq3 = msk_pool.tile([128, FC], bf16, tag="eq3")
                nc.vector.tensor_scalar(eq3[:], yf[:], 3.0, None,
                                        Alu.is_equal)

                et = q_pool.tile([128, C, FC], bf16, tag="et")
                nc.scalar.activation(et[:, 0:2, :], pt[:, 0:2, :], Act.Exp)
                nc.scalar.activation(et[:, 2, :], pt[:, 2, :], Act.Exp)
                nc.scalar.activation(et[:, 3, :], pt[:, 3, :], Act.Exp)

                # Pool picked-logit products (reduced in stage_b)
                pkp = msk_pool.tile([128, 3, FC], bf16, tag="pkp")
                for c in range(3):
                    nc.gpsimd.tensor_tensor(pkp[:, c, :], eqs[c][:],
                                            pt[:, c, :], Alu.mult)
                stash[k, "pkp"] = pkp

                # s3' = e0+e1+e2+BIG*eq3 in PSUM; stage_b later accumulates
                # e3 into the same region (after the Ln(s3') read) to get
                # s4' = s4 + BIG*eq3 without a second accumulation chain.
                s3p = ps_pool.tile([128, FC], f32, tag="s3p")
                for j in range(FC // MMW):
                    js = slice(j * MMW, (j + 1) * MMW)
                    for c in range(3):
                        nc.tensor.matmul(s3p[:, js], ident[:], et[:, c, js],
                                         start=(c == 0), stop=False)
                    nc.tensor.matmul(s3p[:, js], identB[:], eq3[:, js],
                                     start=False, stop=True)
                stash[k, "abe"] = (s3p, et, eqs)

            def stage_b(k):
                # ACT ln/recip, q planes, masked products + reductions
                s3p, et, eqs = stash.pop((k, "abe"))
                lns3 = scr_pool.tile([128, FC], f32, tag="lns3")
                nc.scalar.activation(lns3[:], s3p[:], Act.Ln)
                r3 = q_pool.tile([128, FC], bf16, tag="r3")
                nc.scalar.activation(r3[:], lns3[:], Act.Exp, scale=-1.0)
                # s4' = s3' + e3, accumulated into the same PSUM region (the
                # WAR on the Ln read is tracked by the tile framework)
                for j in range(FC // MMW):
                    js = slice(j * MMW, (j + 1) * MMW)
                    nc.tensor.matmul(s3p[:, js], ident[:], et[:, 3, js],
                                     start=False, stop=True, skip_group_check=True)
                nc.scalar.activation(lns3[:], s3p[:], Act.Ln,
                                     accum_out=col(k, 10))

                qs = []
                for d in range(2):
                    qd = q_pool.tile([128, FC], bf16, tag=f"q{d}")
                    nc.vector.tensor_tensor(qd[:], et[:, d, :], r3[:],
                                            Alu.mult)
                    nc.vector.tensor_scalar(qd[:], qd[:], 1.0, 0.0, Alu.mult,
                                            Alu.add, accum_out=col(k, 3 + d))
                    qs.append(qd)

                for c in range(2):
                    for d in range(2):
                        pr = scr_pool.tile([128, FC], bf16, tag="pr")
                        nc.vector.tensor_tensor(pr[:], eqs[c][:], qs[d][:],
                                                Alu.mult)
                        nc.vector.tensor_scalar(pr[:], pr[:], 1.0, 0.0,
                                                Alu.mult, Alu.add,
                                                accum_out=col(k, 5 + 2 * c + d))

                pkp = stash.pop((k, "pkp"))
                nc.vector.tensor_scalar(pkp[:], pkp[:], 1.0, 0.0, Alu.mult,
                                        Alu.add, accum_out=col(k, 9))

            for k in range(NCHUNK + 1):
                if k < NCHUNK:
                    stage_a(k)
                if k >= 1:
                    stage_b(k - 1)

            nc.sync.dma_start(out=acc_d.ap(), in_=acc[:])
    nc.compile()
    return nc


def _get_nc():
    if "nc" not in _CACHE:
        _CACHE["nc"] = _build()
    return _CACHE["nc"]


def _finalize(acc_list):
    """acc_list: per-core [128, ACC_W] f32 -> scalar loss (f64 host math)."""
    per_sample = []
    for a in acc_list:
        a = a.astype(np.float64).reshape(128, NCHUNK, NQ).sum(axis=1)
        a = a.reshape(B_LOC, HI, NQ).sum(axis=1)  # [32, NQ]
        per_sample.append(a)
    a = np.concatenate(per_sample, axis=0)  # [256, NQ]
    n_c = a[:, 0:3]
    t_d = a[:, 3:5]
    m = a[:, 5:9].reshape(B, 2, 2)
    pk_total = a[:, 9].sum()
    # ceS accumulated ln(s4 + BIG*eq3): subtract the known pad contribution
    npad = L - n_c.sum(axis=1)
    ceS_total = (a[:, 10] - npad * np.log(BIG)).sum()
    ce3_total = 0.0

    mat = np.zeros((B, 3, 3))
    mat[:, :2, :2] = m
    mat[:, 0, 2] = n_c[:, 0] - m[:, 0, 0] - m[:, 0, 1]
    mat[:, 1, 2] = n_c[:, 1] - m[:, 1, 0] - m[:, 1, 1]
    mat[:, 2, 0] = t_d[:, 0] - m[:, 0, 0] - m[:, 1, 0]
    mat[:, 2, 1] = t_d[:, 1] - m[:, 0, 1] - m[:, 1, 1]
    mat[:, 2, 2] = n_c[:, 2] - mat[:, 2, 0] - mat[:, 2, 1]

    j = n_c.sum(axis=1)
    mat /= j[:, None, None]
    det = np.linalg.det(mat)
    dmi = np.where(det < 0, np.log(np.abs(det) + 1e-3),
                   -np.log(np.abs(det) + 1e-3))
    ce = (ceS_total - ce3_total - pk_total) / j.sum()
    loss = DMICE_P * (dmi.sum() / B) + ce
    return np.asarray(loss, dtype=np.float32)


def kernel(pred, labels):
    global LAST_EXEC_NS, LAST_TRACE_PATH
    pred = np.asarray(pred, dtype=np.float32)
    labels = np.asarray(labels, dtype=np.int32)
    assert pred.shape == (B, C, L) and labels.shape == (B, L)
    nc = _get_nc()
    in_maps = [
        {
            "pred": np.ascontiguousarray(pred[i * B_LOC:(i + 1) * B_LOC]),
            "labels": np.ascontiguousarray(labels[i * B_LOC:(i + 1) * B_LOC]),
        }
        for i in range(N_CORES)
    ]
    res = run_bass_kernel_spmd(nc, in_maps, core_ids=list(range(N_CORES)),
                               trace=TRACE)
    LAST_EXEC_NS = res.exec_time_ns
    if res.instructions_and_trace is not None:
        LAST_TRACE_PATH = res.instructions_and_trace[1]
    return _finalize([r["acc"] for r in res.results])


if __name__ == "__main__":
    nc = _build()
    print("build ok")
    from concourse.timeline_sim import TimelineSim
    print("sim ns:", TimelineSim(nc, trace=False).simulate())
